# revision 44
# baseline (speedup 1.0000x reference)
"""Multi-head attention (B=2, S=2048, D=1024, H=16) on 8 TRN2 NeuronCores.

Sharding: core c handles batch c//4 and heads 4*(c%4) .. 4*(c%4)+4
(tensor-parallel over heads x data-parallel over batch).

Single software-pipelined schedule designed to keep the PE (tensor) engine
continuously busy (it is the bottleneck: ~401k matmul cycles ~= 167us at
2.4GHz, vs ~133us of ScalarE exp work):

  - blocks b = 0..7 in (qs, pair)-interleaved order: (0,0),(0,1),(1,0),...
    block i emits scores_i (S^T = kT.T @ qT per head pair, 16 k-blocks),
    exp on ScalarE into pt_i (bf16), and av_{i-1} (A.V of the previous
    block) interleaved per k-block, plus "filler" matmul units (QKV
    projections, O-projections, softmax-denominator replication) woven in
    so the PE never starves while ScalarE chews the exp stream.
  - av_i lags scores_i by exactly one block (pt double-buffered).
  - normalize (1/denominator) for block i is spread over the first k-blocks
    of block i+2; O-projection of q-supertile qs becomes filler in block
    2qs+3 (or the tail for qs=3).
  - X inputs stream through per-tensor double-buffered quarter tiles
    (512 s-positions each); K is loaded once (no reload).
  - ~warmup matmuls at t~2us burn the PE p-state ramp on garbage so real
    work starts at full clock.

Per-core math (all matmuls bf16, fp32 PSUM):
  qT/kT = W @ X^T   [d=256 on partitions, s free]  (transposed projections)
  v     = X @ W^T   [s on partitions, d free] + ones column (softmax den)
  S^T[k,q] = kT.T @ qT -> PSUM, exp(S/8) on ScalarE -> p^T bf16
  out^T[d,q] = sum_k [v|1].T @ p^T  (row 64 = denominator); scale by the
  reciprocal denominator (replicated across partitions via a K=1 ones
  matmul in f32r); partial[s,1024] = out^T.T @ WoT (this core's head block)
Host: full output[b] = sum of the 4 partials for batch b + b_o.
"""
import numpy as np
import ml_dtypes

import concourse.bass as bass
import concourse.mybir as mybir
from concourse.tile import TileContext
from concourse.bass_utils import run_bass_kernel_spmd


def split_multi_waits(nc):
    """This container's walrus codegen allows only one sync-wait command per
    instruction ("Too many sync wait commands" in setupSyncWait). Tile
    sometimes attaches several semaphore waits to one instruction; hoist the
    extras onto dedicated EventSemaphore instructions inserted immediately
    before, on the same engine (sequencers execute in order, so semantics
    are identical)."""
    n = [0]
    for f in nc.m.functions:
        for blk in f.blocks:
            new_insts = []
            changed = False
            for inst in blk.instructions:
                si = inst.sync_info
                if si is not None and len(si.on_wait) > 1:
                    waits = list(si.on_wait)
                    for w in waits[:-1]:
                        n[0] += 1
                        ev = mybir.InstEventSemaphore(
                            name=f"WSPLIT-{n[0]}",
                            ins=[], outs=[],
                        )
                        ev.engine = inst.engine
                        ev.sync_info = mybir.SyncInfo(on_wait=[w], on_update=[])
                        new_insts.append(ev)
                        nc.register_instruction(ev, overwrite=True)
                    inst.sync_info = mybir.SyncInfo(
                        on_wait=[waits[-1]], on_update=list(si.on_update)
                    )
                    changed = True
                new_insts.append(inst)
            if changed:
                blk.instructions = new_insts
    return n[0]

BF16 = mybir.dt.bfloat16
F32 = mybir.dt.float32
F32R = mybir.dt.float32r

B, S, D = 2, 2048, 1024
H, DK = 16, 64
HPC = 4              # heads per core
DC = HPC * DK        # 256 d-model dims per core
N_CORES = 8
P = 128              # partitions
SB = S // P          # 16 s-blocks (k-blocks)
FC = D // P          # 8 feature chunks (contraction tiles)
QSUP = 512           # q supertile width
NQ = S // QSUP       # 4 q supertiles
QTR = 512            # X streaming quarter (s positions)
N_WARMUP = 10        # PE p-state warmup matmuls


def build_bass():
    nc = bass.Bass()
    xtq = nc.dram_tensor("xtq", [D, S], BF16, kind="ExternalInput")
    xtk = nc.dram_tensor("xtk", [D, S], BF16, kind="ExternalInput")
    xtv = nc.dram_tensor("xtv", [D, S], BF16, kind="ExternalInput")
    wqt = nc.dram_tensor("wqt", [D, DC], BF16, kind="ExternalInput")
    wkt = nc.dram_tensor("wkt", [D, DC], BF16, kind="ExternalInput")
    wvt = nc.dram_tensor("wvt", [D, DC], BF16, kind="ExternalInput")
    wot = nc.dram_tensor("wot", [DC, D], BF16, kind="ExternalInput")
    bq = nc.dram_tensor("bq", [DC, 1], F32, kind="ExternalInput")
    bk = nc.dram_tensor("bk", [DC, 1], F32, kind="ExternalInput")
    bvr = nc.dram_tensor("bvr", [1, DC], F32, kind="ExternalInput")
    outp = nc.dram_tensor("outp", [S, D], BF16, kind="ExternalOutput")

    with TileContext(nc) as tc:
        consts = tc.alloc_tile_pool(name="consts", bufs=1)
        qkv = tc.alloc_tile_pool(name="qkv", bufs=1)
        ptpool = tc.alloc_tile_pool(name="ptpool", bufs=2)
        dyn = tc.alloc_tile_pool(name="dyn", bufs=1)
        opool = tc.alloc_tile_pool(name="opool", bufs=8)
        xkp = tc.alloc_tile_pool(name="xkp", bufs=4)
        xqp = tc.alloc_tile_pool(name="xqp", bufs=2)
        xvp = tc.alloc_tile_pool(name="xvp", bufs=2)
        pspool = tc.alloc_tile_pool(name="pspool", bufs=2, space="PSUM")
        avpool = tc.alloc_tile_pool(name="avpool", bufs=1, space="PSUM")

        # ---- persistent SBUF tiles ----
        wk_sb = consts.tile([P, FC, DC], BF16, tag="wk")
        wq_sb = consts.tile([P, FC, DC], BF16, tag="wq")
        wv_sb = consts.tile([P, FC, DC], BF16, tag="wv")
        wo_sb = consts.tile([P, DC // P, D], BF16, tag="wo")
        bq_sb = consts.tile([P, DC // P, 1], F32, tag="bq")
        bk_sb = consts.tile([P, DC // P, 1], F32, tag="bk")
        bv_row = consts.tile([1, DC], F32, tag="bvrow")
        bv_rep = consts.tile([P, DC], F32, tag="bvrep")
        ones_f32 = consts.tile([1, P], F32, tag="ones_f32")
        ones_row = consts.tile([1, P], F32R, tag="ones_row")
        wm_src = consts.tile([1, QSUP], BF16, tag="wm_src")

        qt_sb = qkv.tile([P, DC // P, S], BF16, tag="qt")
        kt_sb = qkv.tile([P, DC // P, S], BF16, tag="kt")
        v_sb = qkv.tile([P, SB, HPC, DK + 1], BF16, tag="v")
        outt_sb = qkv.tile([P, 2, S], BF16, tag="outt")

        # ---- DMA helpers ----
        def dma_w(dst, src_dram, pat):
            nc.sync.dma_start(dst[:], src_dram.rearrange(pat, p=P))

        xq_tiles, xk_tiles, xv_tiles = {}, {}, {}

        def load_quarter(pool, tiles, dram, qtr, nm):
            t = pool.tile([P, FC, QTR], BF16, tag="xq4", name=f"{nm}{qtr}")
            src = dram.rearrange("(c p) s -> p c s", p=P)
            nc.sync.dma_start(t[:], src[:, :, qtr * QTR:(qtr + 1) * QTR])
            tiles[qtr] = t

        # ---- memsets (DVE/Pool, fire immediately) ----
        nc.vector.memset(wm_src[:], 1.0)
        nc.vector.memset(ones_f32[:], 1.0)
        nc.vector.tensor_copy(ones_row[:], ones_f32[:])
        nc.vector.memset(v_sb[:, :, :, DK:], 1.0)

        # ---- upfront DMA queue (order = issue order; paces the kp units) ----
        nc.sync.dma_start(bv_row[:], bvr[:])
        dma_w(wk_sb, wkt, "(c p) d -> p c d")
        dma_w(bk_sb, bk, "(c p) o -> p c o")
        load_quarter(xkp, xk_tiles, xtk, 0, "xk")
        load_quarter(xkp, xk_tiles, xtk, 1, "xk")
        dma_w(wq_sb, wqt, "(c p) d -> p c d")
        dma_w(bq_sb, bq, "(c p) o -> p c o")
        load_quarter(xkp, xk_tiles, xtk, 2, "xk")
        load_quarter(xkp, xk_tiles, xtk, 3, "xk")
        load_quarter(xqp, xq_tiles, xtq, 0, "xq")
        dma_w(wv_sb, wvt, "(c p) d -> p c d")
        load_quarter(xvp, xv_tiles, xtv, 0, "xv")
        load_quarter(xvp, xv_tiles, xtv, 1, "xv")

        # ---- PE warmup: burn the p-state ramp on garbage matmuls ----
        for i in range(N_WARMUP):
            wps = pspool.tile([P, 2, QSUP], F32, tag="ps", name=f"warm{i}")
            nc.tensor.matmul(wps[:, 0, :], wm_src[:, :P], wm_src[:],
                             start=True, stop=True)

        # ---- unit emitters ----
        def bv_unit():
            ps = pspool.tile([P, 2, QSUP], F32, tag="ps", name="bv_ps")
            nc.tensor.matmul(ps[:, 0, :DC], ones_f32[:], bv_row[:],
                             start=True, stop=True)
            nc.vector.tensor_copy(bv_rep[:], ps[:, 0, :DC])

        def kp_unit(dc, ss):
            ps = pspool.tile([P, 2, QSUP], F32, tag="ps", name=f"kp{dc}_{ss}")
            for fc in range(FC):
                nc.tensor.matmul(
                    ps[:, 0, :],
                    wk_sb[:, fc, dc * P:(dc + 1) * P],
                    xk_tiles[ss][:, fc, :],
                    start=(fc == 0), stop=(fc == FC - 1),
                )
            nc.vector.tensor_scalar(
                kt_sb[:, dc, ss * QTR:(ss + 1) * QTR],
                ps[:, 0, :], bk_sb[:, dc, :], None, mybir.AluOpType.add,
            )

        def psum_tile(tag, name):
            pool = pspool if tag == "ps" else avpool
            return pool.tile([P, 2, QSUP], F32, tag=tag, name=name)

        def qp_unit(dc, ss, tag="ps"):
            ps = psum_tile(tag, f"qp{dc}_{ss}")
            for fc in range(FC):
                nc.tensor.matmul(
                    ps[:, 0, :],
                    wq_sb[:, fc, dc * P:(dc + 1) * P],
                    xq_tiles[ss][:, fc, :],
                    start=(fc == 0), stop=(fc == FC - 1),
                )
            nc.vector.tensor_scalar(
                qt_sb[:, dc, ss * QTR:(ss + 1) * QTR],
                ps[:, 0, :], bq_sb[:, dc, :], None, mybir.AluOpType.add,
            )

        def vp_unit(sb_i, tag="ps"):
            qtr, off = sb_i // 4, (sb_i % 4) * P
            ps = psum_tile(tag, f"vp{sb_i}")
            for fc in range(FC):
                nc.tensor.matmul(
                    ps[:, 0, :DC],
                    xv_tiles[qtr][:, fc, off:off + P],
                    wv_sb[:, fc, :],
                    start=(fc == 0), stop=(fc == FC - 1),
                )
            nc.vector.tensor_tensor(
                v_sb[:, sb_i, :, :DK],
                ps[:, 0, :DC].rearrange("p (h d) -> p h d", h=HPC),
                bv_rep[:].rearrange("p (h d) -> p h d", h=HPC),
                mybir.AluOpType.add,
            )

        def scores_unit(pt, pair, qs, kb):
            q0 = qs * QSUP
            sc = pspool.tile([P, 2, QSUP], F32, tag="ps", name=f"sc{pair}{qs}_{kb}")
            for hh in range(2):
                hp = hh * DK
                nc.tensor.matmul(
                    sc[:, hh, :],
                    kt_sb[hp:hp + DK, pair, kb * P:(kb + 1) * P],
                    qt_sb[hp:hp + DK, pair, q0:q0 + QSUP],
                    start=True, stop=True,
                )
            nc.scalar.activation(
                pt[:, :, kb, :], sc[:],
                mybir.ActivationFunctionType.Exp,
                bias=0.0, scale=0.125,
            )

        # av accumulator for block i: allocated at first-write time (start of
        # block i+1) with a parity tag, so each block's idle av slot can host
        # filler psums without FIFO hazards
        def av_tile(i):
            return avpool.tile([DK + 1, 2, QSUP], F32, tag=f"av{i % 2}",
                               name=f"av_{i}")

        def av_unit(av_ps, pt, pair, kb):
            for hh in range(2):
                h = 2 * pair + hh
                nc.tensor.matmul(
                    av_ps[:, hh, :],
                    v_sb[:, kb, h, :],
                    pt[:, hh, kb, :],
                    start=(kb == 0), stop=(kb == SB - 1),
                )

        # normalize: per-head pipeline spread across the following block
        def norm_s1(av_ps, i):
            # reciprocal of both heads' denominators, written as f32r so the
            # replicate matmul runs at 1 cycle/row
            rec = dyn.tile([1, 2, QSUP], F32R, tag="rec", name=f"rec{i}")
            with nc.allow_low_precision(reason="f32r is f32-width"):
                nc.vector.reciprocal(rec[:], av_ps[DK:, :, :])
            return rec

        def norm_s2h(rec_r, hh, i):
            rps = pspool.tile([P, QSUP], F32, tag="ps", name=f"rp{i}_{hh}")
            nc.tensor.matmul(rps[:], ones_row[:], rec_r[:, hh, :],
                             start=True, stop=True)
            return rps

        def norm_s3h(rps, rec_rep, hh, eng="dve"):
            if eng == "act":
                nc.scalar.activation(rec_rep[:, hh, :], rps[:],
                                     mybir.ActivationFunctionType.Copy)
            else:
                nc.vector.tensor_copy(rec_rep[:, hh, :], rps[:])

        def norm_s4h(av_ps, rec_rep, pair, qs, hh):
            q0 = qs * QSUP
            nc.vector.tensor_tensor(
                outt_sb[hh * DK:(hh + 1) * DK, pair, q0:q0 + QSUP],
                av_ps[:DK, hh, :],
                rec_rep[hh * DK:(hh + 1) * DK, hh, :],
                mybir.AluOpType.mult,
            )

        def rep_tile(i):
            return dyn.tile([P, 2, QSUP], F32, tag="rec_rep", name=f"rrep{i}")

        def oproj_unit(qs, j, nk, copy_eng="dve", ps_tag="ps"):
            sb_i = qs * 4 + j
            ps = psum_tile(ps_tag, f"op{sb_i}_{nk}")
            for pair in range(2):
                nc.tensor.matmul(
                    ps[:, 0, :],
                    outt_sb[:, pair, sb_i * P:(sb_i + 1) * P],
                    wo_sb[:, pair, nk * QSUP:(nk + 1) * QSUP],
                    start=(pair == 0), stop=(pair == 1),
                )
            o_sb = opool.tile([P, QSUP], BF16, tag="o", bufs=4,
                                  name=f"o{sb_i}_{nk}")
            if copy_eng == "act":
                nc.scalar.activation(o_sb[:], ps[:, 0, :],
                                     mybir.ActivationFunctionType.Copy)
            else:
                nc.vector.tensor_copy(o_sb[:], ps[:, 0, :])
            nc.sync.dma_start(
                outp[sb_i * P:(sb_i + 1) * P, nk * QSUP:(nk + 1) * QSUP],
                o_sb[:],
            )

        def op_split(qs, j, nk, tag):
            """oproj unit split into two 1-matmul stages (finer PE filler)."""
            sb_i = qs * 4 + j
            st = {}

            def open_():
                st["ps"] = psum_tile(tag, f"op{sb_i}_{nk}")
                nc.tensor.matmul(
                    st["ps"][:, 0, :],
                    outt_sb[:, 0, sb_i * P:(sb_i + 1) * P],
                    wo_sb[:, 0, nk * QSUP:(nk + 1) * QSUP],
                    start=True, stop=False,
                )

            def close_():
                ps = st["ps"]
                nc.tensor.matmul(
                    ps[:, 0, :],
                    outt_sb[:, 1, sb_i * P:(sb_i + 1) * P],
                    wo_sb[:, 1, nk * QSUP:(nk + 1) * QSUP],
                    start=False, stop=True,
                )
                o_sb = opool.tile([P, QSUP], BF16, tag="o", bufs=4,
                                  name=f"o{sb_i}_{nk}")
                nc.vector.tensor_copy(o_sb[:], ps[:, 0, :])
                nc.sync.dma_start(
                    outp[sb_i * P:(sb_i + 1) * P, nk * QSUP:(nk + 1) * QSUP],
                    o_sb[:],
                )
            return open_, close_

        def qp_split(dc, ss, tag):
            """q-projection split into two 4-matmul stages."""
            st = {}

            def open_():
                st["ps"] = psum_tile(tag, f"qp{dc}_{ss}")
                for fc in range(4):
                    nc.tensor.matmul(
                        st["ps"][:, 0, :],
                        wq_sb[:, fc, dc * P:(dc + 1) * P],
                        xq_tiles[ss][:, fc, :],
                        start=(fc == 0), stop=False,
                    )

            def close_():
                ps = st["ps"]
                for fc in range(4, FC):
                    nc.tensor.matmul(
                        ps[:, 0, :],
                        wq_sb[:, fc, dc * P:(dc + 1) * P],
                        xq_tiles[ss][:, fc, :],
                        start=False, stop=(fc == FC - 1),
                    )
                nc.vector.tensor_scalar(
                    qt_sb[:, dc, ss * QTR:(ss + 1) * QTR],
                    ps[:, 0, :], bq_sb[:, dc, :], None, mybir.AluOpType.add,
                )
            return open_, close_

        # ---- pre-loop: all K/Q projections run upfront, paced by the DMA
        # stream (kq0..kq3, qq0); the PE is busy from first arrival on ----
        bv_unit()
        for ss in range(4):
            kp_unit(0, ss)
            kp_unit(1, ss)
        qp_unit(0, 0)
        qp_unit(1, 0)

        # ---- block schedule: (qs, pair)-interleaved ----
        BLOCKS = [(qs, pair) for qs in range(NQ) for pair in range(2)]

        # filler units per block: {kb: [emit_fn]}. Fillers in block i use the
        # av-parity slot freed by norm_{i-2} (tag av{i%2}) from kb8 on.
        def op_fill(qs, ks, tag):
            return [lambda j=k // 2, nk=k % 2: oproj_unit(qs, j, nk, "dve", tag)
                    for k in ks]

        def F(i):
            t = f"av{i % 2}"
            if i == 0:
                # v projections in pairs on even kbs (vp 14,15 -> block 1);
                # both av slots are free here — alternate tags
                return {kb: [lambda s=kb: vp_unit(s, "av0"),
                             lambda s=kb + 1: vp_unit(s, "av1")]
                        for kb in range(0, 14, 2)}
            if i == 1:
                return {1: [lambda: vp_unit(14, t)],
                        2: [lambda: qp_unit(0, 1, t)],
                        3: [lambda: vp_unit(15, t)],
                        6: [lambda: qp_unit(1, 1, t)]}
            if i == 2:
                aO, aC = qp_split(0, 2, t)
                bO, bC = qp_split(1, 2, t)
                return {9: [aO], 10: [aC], 11: [bO], 12: [bC]}
            if i == 3:
                sp = [op_split(0, k // 2, k % 2, t) for k in range(3)]
                qO, qC = qp_split(0, 3, "ps")
                return {1: [qO], 2: [qC],
                        9: [sp[0][0]], 10: [sp[0][1]], 11: [sp[1][0]],
                        12: [sp[1][1]], 13: [sp[2][0]], 14: [sp[2][1]],
                        15: [lambda: oproj_unit(0, 1, 1, "dve", t)]}
            if i == 4:
                sp = [op_split(0, 2 + k // 2, k % 2, t) for k in range(3)]
                qO, qC = qp_split(1, 3, "ps")
                return {1: [qO], 2: [qC],
                        9: [sp[0][0]], 10: [sp[0][1]], 11: [sp[1][0]],
                        12: [sp[1][1]], 13: [sp[2][0]], 14: [sp[2][1]]}
            if i == 5:
                sp = [op_split(1, k // 2, k % 2, t) for k in range(3)]
                eO, eC = op_split(0, 3, 1, "ps")
                return {1: [eO], 2: [eC],
                        9: [sp[0][0]], 10: [sp[0][1]], 11: [sp[1][0]],
                        12: [sp[1][1]], 13: [sp[2][0]], 14: [sp[2][1]],
                        15: [lambda: oproj_unit(1, 1, 1, "dve", t)]}
            if i == 6:
                sp = [op_split(1, 2 + k // 2, k % 2, t) for k in range(3)]
                return {9: [sp[0][0]], 10: [sp[0][1]], 11: [sp[1][0]],
                        12: [sp[1][1]], 13: [sp[2][0]], 14: [sp[2][1]],
                        15: [lambda: oproj_unit(1, 3, 1, "dve", t)]}
            if i == 7:
                return {}
            return {}

        # per-block DMA emission points: block -> {kb: [emit_fn]}
        DMAS = {
            0: {2: [lambda: load_quarter(xvp, xv_tiles, xtv, 2, "xv")],
                4: [lambda: load_quarter(xqp, xq_tiles, xtq, 1, "xq")],
                6: [lambda: load_quarter(xvp, xv_tiles, xtv, 3, "xv")],
                12: [lambda: load_quarter(xqp, xq_tiles, xtq, 2, "xq")],
                13: [lambda: dma_w(wo_sb, wot, "(c p) n -> p c n")]},
            2: {0: [lambda: load_quarter(xqp, xq_tiles, xtq, 3, "xq")]},
        }

        prev = None         # (av_ps, pt, pair, qs) of block i-1
        norm_q = []         # pending normalize state machines
        navs = {}           # block idx -> av_ps (for tail norms)

        for i, (qs, pair) in enumerate(BLOCKS):
            pt = ptpool.tile([P, 2, SB, QSUP], BF16, tag="pt", name=f"pt{i}")
            if prev is not None:
                # av accumulator for block i-1 (first written in this block)
                pav = av_tile(i - 1)
                navs[i - 1] = pav
                prev = (pav,) + prev[1:]
            fillers = F(i)
            dmas = DMAS.get(i, {})
            norm_st = norm_q.pop(0) if norm_q else None
            nstate = {}
            last = (i == 7)
            av7 = None
            av7_sched = ([[]] * 8 + [[0, 1], [2, 3], [4, 5], [6, 7],
                         [8, 9], [10, 11], [12, 13], [14, 15]]) if last else None
            # av_{i-1} starts at kb2 (clear of the previous block's last
            # filler copy on the same psum tag) and rebalances over the block
            if last:
                avp_sched = [[], [], [0, 1, 2], [3, 4, 5], [6, 7, 8],
                             [9, 10, 11], [12, 13], [14, 15]] + [[]] * 8
            else:
                avp_sched = ([[], [], [0, 1]] + [[k] for k in range(2, 14)]
                             + [[14, 15]])
            for kb in range(SB):
                for fn in dmas.get(kb, []):
                    fn()
                scores_unit(pt, pair, qs, kb)
                if prev is not None:
                    pav, ppt, ppair, pqs = prev
                    for k in avp_sched[kb]:
                        av_unit(pav, ppt, ppair, k)
                if norm_st is not None:
                    nav, npair, nqs, nidx = norm_st
                    sched = ((1, 2, 3, 4, 5, 6, 7) if last
                             else (1, 3, 4, 5, 6, 7, 8))
                    if kb == sched[0]:
                        nstate["rec"] = norm_s1(nav, nidx)
                    elif kb == sched[1]:
                        nstate["rpA"] = norm_s2h(nstate["rec"], 0, nidx)
                        nstate["rep"] = rep_tile(nidx)
                    elif kb == sched[2]:
                        norm_s3h(nstate["rpA"], nstate["rep"], 0)
                    elif kb == sched[3]:
                        norm_s4h(nav, nstate["rep"], npair, nqs, 0)
                    elif kb == sched[4]:
                        nstate["rpB"] = norm_s2h(nstate["rec"], 1, nidx)
                    elif kb == sched[5]:
                        norm_s3h(nstate["rpB"], nstate["rep"], 1)
                    elif kb == sched[6]:
                        norm_s4h(nav, nstate["rep"], npair, nqs, 1)
                for fn in fillers.get(kb, []):
                    fn()
                if last:
                    # norm_6 compressed into kb10..15 (av_6 finished by kb8)
                    nav6, (nq6, np6) = navs[6], BLOCKS[6]
                    if kb == 10:
                        nstate["rec6"] = norm_s1(nav6, 6)
                    elif kb == 11:
                        nstate["rpA6"] = norm_s2h(nstate["rec6"], 0, 6)
                        nstate["rep6"] = rep_tile(6)
                    elif kb == 12:
                        norm_s3h(nstate["rpA6"], nstate["rep6"], 0)
                    elif kb == 13:
                        nstate["rpB6"] = norm_s2h(nstate["rec6"], 1, 6)
                    elif kb == 14:
                        norm_s3h(nstate["rpB6"], nstate["rep6"], 1)
                        norm_s4h(nav6, nstate["rep6"], np6, nq6, 0)
                    elif kb == 15:
                        norm_s4h(nav6, nstate["rep6"], np6, nq6, 1)
                    # av_7 chases exp_7 (starts kb8, after norm_5 frees av1)
                    if kb == 8:
                        av7 = av_tile(7)
                        navs[7] = av7
                    for k in av7_sched[kb]:
                        av_unit(av7, pt, pair, k)
            if prev is not None and not last:
                pav, ppair, pqs = prev[0], prev[2], prev[3]
                norm_q.append((pav, ppair, pqs, i - 1))
            prev = (navs.get(7), pt, pair, qs) if last else (None, pt, pair, qs)

        # ---- tail: oproj(qs2) + norm_7 + oproj(qs3); output chunks are
        # batched into 2-s-block staging tiles (one DMA each, avoiding 16
        # serialized HWDGE issues at the very end) ----
        def tail_op(qs, j, nk, obig, slot, copy_eng, ps_tag):
            sb_i = qs * 4 + j
            ps = psum_tile(ps_tag, f"op{sb_i}_{nk}")
            for pair in range(2):
                nc.tensor.matmul(
                    ps[:, 0, :],
                    outt_sb[:, pair, sb_i * P:(sb_i + 1) * P],
                    wo_sb[:, pair, nk * QSUP:(nk + 1) * QSUP],
                    start=(pair == 0), stop=(pair == 1),
                )
            if copy_eng == "act":
                nc.scalar.activation(obig[:, slot, nk, :], ps[:, 0, :],
                                     mybir.ActivationFunctionType.Copy)
            else:
                nc.vector.tensor_copy(obig[:, slot, nk, :], ps[:, 0, :])

        pav, ppt, ppair, pqs = prev
        ob = outp.rearrange("(b p) (nk s) -> p b nk s", p=P, nk=2)

        def obig_tile(nm):
            return opool.tile([P, 2, 2, QSUP], BF16, tag="obig", bufs=2,
                              name=nm)

        # qs2 o-projections overlap the norm_7 chain (per-head reciprocal
        # so the h0 branch starts replicating immediately)
        rec7 = dyn.tile([1, 2, QSUP], F32R, tag="rec", name="rec7")
        with nc.allow_low_precision(reason="f32r is f32-width"):
            nc.vector.reciprocal(rec7[:, 0, :], pav[DK:, 0, :])
        obigA = obig_tile("obigA")
        tail_op(2, 0, 0, obigA, 0, "act", "ps")
        rep7 = rep_tile(7)
        rpA7 = norm_s2h(rec7, 0, 7)
        with nc.allow_low_precision(reason="f32r is f32-width"):
            nc.vector.reciprocal(rec7[:, 1, :], pav[DK:, 1, :])
        tail_op(2, 0, 1, obigA, 0, "dve", "av0")
        norm_s3h(rpA7, rep7, 0, "act")
        tail_op(2, 1, 0, obigA, 1, "act", "ps")
        rpB7 = norm_s2h(rec7, 1, 7)
        norm_s4h(pav, rep7, ppair, pqs, 0)
        tail_op(2, 1, 1, obigA, 1, "dve", "av0")
        nc.sync.dma_start(ob[:, 8:10], obigA[:])
        obigB = obig_tile("obigB")
        tail_op(2, 2, 0, obigB, 0, "act", "ps")
        norm_s3h(rpB7, rep7, 1, "dve")
        tail_op(2, 2, 1, obigB, 0, "dve", "av0")
        norm_s4h(pav, rep7, ppair, pqs, 1)
        tail_op(2, 3, 0, obigB, 1, "act", "ps")
        tail_op(2, 3, 1, obigB, 1, "dve", "av0")
        nc.sync.dma_start(ob[:, 10:12], obigB[:])

        # qs3 o-projections (all gated on norm_7)
        TAGS3 = ["ps", "av0", "av1", "ps", "av0", "av1", "ps", "av0"]
        obig1 = obig_tile("obig1")
        obig2 = opool.tile([P, 1, 2, QSUP], BF16, tag="obig2", bufs=2,
                           name="obig2")
        obig3 = opool.tile([P, 1, 2, QSUP], BF16, tag="obig2", bufs=2,
                           name="obig3")
        for k in range(8):
            if k < 4:
                big, slot = obig1, k // 2
            else:
                big, slot = (obig2, 0) if k < 6 else (obig3, 0)
            tail_op(3, k // 2, k % 2, big, slot,
                    "act" if k % 2 else "dve", TAGS3[k])
            if k == 3:
                nc.sync.dma_start(ob[:, 12:14], obig1[:])
            elif k == 5:
                nc.sync.dma_start(ob[:, 14:15], obig2[:])
        nc.sync.dma_start(ob[:, 15:16], obig3[:])

        for pool in (avpool, pspool, xvp, xqp, xkp, opool, dyn,
                     ptpool, qkv, consts):
            pool.release()

    split_multi_waits(nc)
    return nc


_NC_CACHE = None


def prep_in_maps(Q, K, V, W_q, b_q, W_k, b_k, W_v, b_v, W_o, b_o):
    """Host-side sharding: per-core input dicts (transposed, bf16-cast)."""
    bf = ml_dtypes.bfloat16
    Q, K, V = np.asarray(Q), np.asarray(K), np.asarray(V)
    xt = {}   # per batch: transposed bf16 inputs
    for b in range(B):
        xt[b] = (
            np.ascontiguousarray(Q[b].T).astype(bf),
            np.ascontiguousarray(K[b].T).astype(bf),
            np.ascontiguousarray(V[b].T).astype(bf),
        )
    in_maps = []
    for c in range(N_CORES):
        b = c // 4
        g = c % 4
        sl = slice(g * DC, (g + 1) * DC)
        in_maps.append({
            "xtq": xt[b][0], "xtk": xt[b][1], "xtv": xt[b][2],
            "wqt": np.ascontiguousarray(np.asarray(W_q)[sl, :].T).astype(bf),
            "wkt": np.ascontiguousarray(np.asarray(W_k)[sl, :].T).astype(bf),
            "wvt": np.ascontiguousarray(np.asarray(W_v)[sl, :].T).astype(bf),
            "wot": np.ascontiguousarray(np.asarray(W_o)[:, sl].T).astype(bf),
            "bq": np.asarray(b_q)[sl].reshape(DC, 1).astype(np.float32),
            "bk": np.asarray(b_k)[sl].reshape(DC, 1).astype(np.float32),
            "bvr": np.asarray(b_v)[sl].reshape(1, DC).astype(np.float32),
        })
    return in_maps


def gather_out(partials, b_o):
    """Host-side unshard: sum the four W_o-row partials per batch + b_o."""
    out = np.zeros((B, S, D), np.float32)
    for c in range(N_CORES):
        out[c // 4] += np.asarray(partials[c]).astype(np.float32)
    out += np.asarray(b_o).astype(np.float32)
    return out


def kernel(Q, K, V, W_q, b_q, W_k, b_k, W_v, b_v, W_o, b_o):
    global _NC_CACHE
    in_maps = prep_in_maps(Q, K, V, W_q, b_q, W_k, b_k, W_v, b_v, W_o, b_o)
    if _NC_CACHE is None:
        _NC_CACHE = build_bass()
    res = run_bass_kernel_spmd(_NC_CACHE, in_maps, core_ids=list(range(N_CORES)))
    return gather_out([res.results[c]["outp"] for c in range(N_CORES)], b_o)
